# revision 5
# baseline (speedup 1.0000x reference)
"""CRF loss kernel for Trainium2 (8 NeuronCores, data-parallel over batch).

Math: the log-domain forward recurrence
    alpha_t[i] = logsumexp_j(alpha_{t-1}[j] + trans[i,j]) + feat_t[i]
is run in probability domain:
    P_t = exp(feat_t - c) * (E @ P_{t-1}),   E = exp(trans)
so each step is one tiny matmul plus one VectorE multiply.

Layout: T=64 tags use only half the 128 SBUF partitions, and the cost of a
DVE/matmul op depends only on its FREE size, so two 32-column batch groups
are STACKED on the partition axis (partitions 0:64 = cols 0:16, 64:128 =
cols 16:32 of the group) and advanced by a block-diagonal 128x128 transition
matrix. One matmul + one DVE multiply then serve 32 batch columns at free
width 16. Two such superchains interleave; steady state is DVE-bound at
~284ns/step = 2 x (125ns PSUM-access bubble + 16x1.04ns), just under the
serial link latency (~100ns matmul visibility + 45ns DVE decode + DVE busy).

All matmul operands are bf16 (error budget is huge: tolerance 2e-2 on a
loss of magnitude ~1e3; bf16 shares fp32's exponent range). A constant
shift c plus a per-superchain renorm every 128 steps keeps P in range;
renorms are staggered across superchains, measured off the critical path
(TensorE 2-row ones-matmul + DVE reciprocal), broadcast back to 128 rows
via a second matmul (2-partition stationary), copied to SBUF by the Act
engine, and folded into the exp(feat) tile of a later step by the Pool
engine. The STOP rows (partitions 63 and 127) of the state history are
archived in SBUF, streamed to DRAM in blocks behind the chain, and the
host epilogue picks slot seq_len[b]+1 per batch column.
"""
import numpy as np

_B, _S, _T = 512, 512, 64
_NCORE = 8
_BC = _B // _NCORE          # 64 batches per core
_G = 2                      # superchains per core
_WG = _BC // _G             # 32 batch columns per superchain
_R2 = 2                     # column groups stacked per superchain
_W = _WG // _R2             # 16 free columns per superchain tile
_P = 128                    # partitions
_START, _STOP = 62, 63
_R = 128                    # renorm period (per superchain)
_LAG = 4                    # renorm measured at t applies at t+_LAG
_NSTEP = _S + 1             # matmul steps 1..513
_NHIST = _NSTEP + 1         # history slots 0..513 (slot 0 unused)
# superchain g renorms at t = _R*(k+1) + 8*g: staggered so aux tiles don't
# coexist
_RENORM_TG = [[_R * (k + 1) + 8 * g for k in range(3)] for g in range(_G)]
_NEVT = 3
_CH = 8                     # steps per feat DMA/exp chunk
_HBLK = 4 * _CH             # hist63 streaming block (steps)

# boot column layout: [E2T | ones2 | Bm | featT2 steps 0..2]
_BOOT_E = 0                  # E2T at cols 0:128
_BOOT_ONES = _P              # ones2 at cols 128:130
_BOOT_F = _P + 2             # featT2[:, 0:96] at cols 130:226
_BOOT_COLS = _BOOT_F + 3 * _G * _W

_cache = {}


def _build_nc():
    import concourse.bass as bass
    import concourse.bacc as bacc
    import concourse.tile as tile
    from concourse import mybir
    from contextlib import ExitStack

    f32 = mybir.dt.float32
    bf16 = mybir.dt.bfloat16
    GW = _G * _W            # 32 = feature columns per step
    nc = bacc.Bacc("TRN2", target_bir_lowering=False, debug=False,
                   num_devices=_NCORE)
    # featT2 block m (m>=1) is step m's features; block 0 is p0.
    featT2 = nc.dram_tensor("featT2", [_P, (_NSTEP + 1) * GW], bf16,
                            kind="ExternalInput").ap()
    boot = nc.dram_tensor("boot", [_P, _BOOT_COLS], bf16,
                          kind="ExternalInput").ap()
    bootf = nc.dram_tensor("bootf", [_R2, _P], f32,
                           kind="ExternalInput").ap()
    hist63o = [nc.dram_tensor(f"hist63_{g}", [_R2, _NHIST * _W], bf16,
                              kind="ExternalOutput").ap() for g in range(_G)]
    sinvo = [nc.dram_tensor(f"sinv_{g}", [_R2, _NEVT * _W], f32,
                            kind="ExternalOutput").ap() for g in range(_G)]

    with tile.TileContext(nc) as tc, ExitStack() as ctx:
        consts = ctx.enter_context(tc.tile_pool(name="consts", bufs=1))
        fpool = ctx.enter_context(tc.tile_pool(name="fpool", bufs=3))
        epool = ctx.enter_context(tc.tile_pool(name="epool", bufs=3))
        ps_g = [ctx.enter_context(
            tc.tile_pool(name=f"ps{g}", bufs=2, space="PSUM"))
            for g in range(_G)]
        ps_aux = ctx.enter_context(tc.tile_pool(name="ps_aux", bufs=2,
                                                space="PSUM"))
        bcpool = ctx.enter_context(tc.tile_pool(name="bcpool", bufs=2))

        # one boot DMA delivers E2T, ones2, Bm, p0, and steps 1-2's features
        boot_sb = consts.tile([_P, _BOOT_COLS], bf16, name="boot_sb")
        nc.sync.dma_start(boot_sb[:, :], boot)
        E2_sb = boot_sb[:, _BOOT_E:_BOOT_E + _P]
        ones2 = boot_sb[:, _BOOT_ONES:_BOOT_ONES + _R2]
        Bm_sb = consts.tile([_R2, _P], f32, name="Bm_sb")
        nc.sync.dma_start(Bm_sb[:, :], bootf)
        Bm = Bm_sb[:, :]

        hists = [consts.tile([_P, _NHIST * _W], bf16, name=f"hist{g}")
                 for g in range(_G)]
        sinvs = [consts.tile([_R2, _NEVT * _W], f32, name=f"sinv{g}")
                 for g in range(_G)]

        renorm_at = {}
        fold_at = {}
        for g in range(_G):
            for e, te in enumerate(_RENORM_TG[g]):
                renorm_at[(te, g)] = e
                fold_at[(te + _LAG, g)] = e
        last_evt_t = max(te for g in range(_G) for te in _RENORM_TG[g])

        curs = [boot_sb[:, _BOOT_F + g * _W:_BOOT_F + (g + 1) * _W]
                for g in range(_G)]
        evt_bc = {}
        sinv_sent = False
        hist_sent = [_W] * _G            # hist63 cols streamed (slot 0 unused)
        t = 1
        while t <= _NSTEP:
            if t == 1:
                n_t = 2
                fch_f = boot_sb[:, _BOOT_F + GW:]
            else:
                n_t = min(_CH, _NSTEP - t + 1)
                fchunk = fpool.tile([_P, _CH * GW], bf16, tag="fchunk")
                nc.sync.dma_start(
                    fchunk[:, : n_t * GW],
                    featT2[:, t * GW: (t + n_t) * GW],
                )
                fch_f = fchunk[:, : n_t * GW]
            Fch = epool.tile([_P, _CH * GW], bf16, tag="Fch")
            nc.scalar.activation(
                Fch[:, : n_t * GW], fch_f,
                mybir.ActivationFunctionType.Exp,
            )
            for k in range(n_t):
                for g in range(_G):
                    fsl = Fch[:, k * GW + g * _W: k * GW + (g + 1) * _W]
                    if (t, g) in fold_at:
                        # apply the pending renorm scale to this step's F
                        bc = evt_bc.pop((fold_at[(t, g)], g))
                        nc.gpsimd.tensor_mul(fsl, fsl, bc[:, :])
                    ps = ps_g[g].tile([_P, _W], f32, tag=f"ps{g}")
                    nc.tensor.matmul(ps[:, :], E2_sb, curs[g],
                                     start=True, stop=True)
                    dst = hists[g][:, t * _W: (t + 1) * _W]
                    nc.vector.tensor_mul(dst, ps[:, :], fsl)
                    curs[g] = dst
                    if (t, g) in renorm_at:
                        e = renorm_at[(t, g)]
                        s_ps = ps_aux.tile([_R2, _W], f32, tag="s_ps")
                        nc.tensor.matmul(s_ps[:, :], ones2, dst,
                                         start=True, stop=True)
                        s_sb = bcpool.tile([_R2, _W], f32, tag="s_sb")
                        nc.scalar.copy(s_sb[:, :], s_ps[:, :])
                        sv = sinvs[g][:, e * _W: (e + 1) * _W]
                        nc.vector.reciprocal(sv, s_sb[:, :])
                        bc_ps = ps_aux.tile([_P, _W], f32, tag="bc_ps")
                        nc.tensor.matmul(bc_ps[:, :], Bm, sv,
                                         start=True, stop=True)
                        bc = bcpool.tile([_P, _W], f32, tag="bc")
                        nc.scalar.copy(bc[:, :], bc_ps[:, :])
                        evt_bc[(e, g)] = bc
                t += 1
            # stream completed hist63 blocks out behind the chain; flush every
            # chunk near the end so the final post-chain DMA is tiny
            if t - hist_sent[0] // _W > _HBLK or t > _NSTEP - 2 * _CH:
                for g in range(_G):
                    lo, hi = hist_sent[g], t * _W
                    # hist63 streaming rides the idle Pool/Act DGE queues so
                    # the SP queue stays exclusive to feature chunks
                    eng = nc.scalar if (t > _NSTEP and g == 0) else nc.gpsimd
                    eng.dma_start(hist63o[g][0:1, lo:hi],
                                  hists[g][_STOP:_STOP + 1, lo:hi])
                    eng.dma_start(hist63o[g][1:2, lo:hi],
                                  hists[g][_T + _STOP:_T + _STOP + 1, lo:hi])
                    hist_sent[g] = hi
            if t > last_evt_t and not sinv_sent:
                sinv_sent = True
                for g in range(_G):
                    nc.sync.dma_start(sinvo[g], sinvs[g][:, :])
    nc.compile()
    return nc


def _prep_inputs(feas, transitions):
    import ml_dtypes

    E = np.exp(transitions.astype(np.float32))
    rows = np.ones(_T, bool)
    rows[_START] = False
    c = float(np.log(E.sum(1)[rows]).mean())
    ET = np.ascontiguousarray(E.T).astype(np.float32)  # ET[j,i]=E[i,j]
    E2T = np.zeros((_P, _P), np.float32)
    E2T[:_T, :_T] = ET
    E2T[_T:, _T:] = ET
    ones2 = np.zeros((_P, _R2), np.float32)
    ones2[:_T, 0] = 1.0
    ones2[_T:, 1] = 1.0
    Bpad = np.zeros((_P, _P), np.float32)       # rows 0:2 hold Bm
    Bpad[0, :_T] = 1.0
    Bpad[1, _T:] = 1.0

    # featT2 per core: [P, (1+NSTEP)*GW]; block 0 = p0 (stacked one-hot at
    # START), block m>=1 is step m's features shifted by -c; step 513 -> -c.
    ft = np.transpose(feas.astype(np.float32), (2, 1, 0)) - np.float32(c)  # [T,S,B]
    GW = _G * _W
    in_maps = []
    for cix in range(_NCORE):
        sl = ft[:, :, cix * _BC: (cix + 1) * _BC]                  # [T,S,BC]
        full = np.empty((_T, _NSTEP + 1, _BC), np.float32)
        p0 = np.zeros((_T, _BC), np.float32)
        p0[_START, :] = 1.0
        full[:, 0, :] = p0
        full[:, 1:_S + 1, :] = sl
        full[:, _S + 1, :] = -c
        # stack: featT2[p, m*GW + g*W + n] = full[p%T, m, g*WG + (p//T)*W + n]
        fu = full.reshape(_T, _NSTEP + 1, _G, _R2, _W)             # [T,m,g,r,n]
        featT2 = np.concatenate([fu[:, :, :, 0, :], fu[:, :, :, 1, :]],
                                axis=0)                            # [P,m,g,n]
        featT2 = np.ascontiguousarray(
            featT2.reshape(_P, (_NSTEP + 1) * GW)).astype(ml_dtypes.bfloat16)
        boot = np.hstack([
            E2T, ones2, featT2[:, 0:3 * GW].astype(np.float32),
        ]).astype(ml_dtypes.bfloat16)
        in_maps.append({
            "featT2": featT2,
            "boot": np.ascontiguousarray(boot),
            "bootf": np.ascontiguousarray(Bpad[0:_R2, :]),
        })
    return c, in_maps


def kernel(feas, transitions, tag, seq_len):
    from concourse.bass_utils import run_bass_kernel_spmd

    feas = np.asarray(feas)
    transitions = np.asarray(transitions)
    tag = np.asarray(tag)
    seq_len = np.asarray(seq_len)

    if "nc" not in _cache:
        _cache["nc"] = _build_nc()
    nc = _cache["nc"]

    c, in_maps = _prep_inputs(feas, transitions)
    res = run_bass_kernel_spmd(nc, in_maps, list(range(_NCORE))).results

    # ---- host epilogue: norm from archived history ----
    L = seq_len.astype(np.int64)                                        # [B]
    # batch column b = cix*BC + g*WG + r*W + n
    hist63 = np.concatenate(
        [res[cix][f"hist63_{g}"].reshape(_R2, _NHIST, _W)
         .transpose(1, 0, 2).reshape(_NHIST, _WG).astype(np.float64)
         for cix in range(_NCORE) for g in range(_G)], axis=1
    )                                                                   # [NHIST, B]
    sinv = np.concatenate(
        [res[cix][f"sinv_{g}"].reshape(_R2, _NEVT, _W)
         .transpose(1, 0, 2).reshape(_NEVT, _WG).astype(np.float64)
         for cix in range(_NCORE) for g in range(_G)], axis=1
    )                                                                   # [NEVT, B]
    tevt = np.concatenate(
        [np.asarray(_RENORM_TG[g])[:, None].repeat(_WG, 1)
         for _ in range(_NCORE) for g in range(_G)], axis=1
    )                                                                   # [NEVT, B]
    # scale 1/s_e is folded into F of step t_e+_LAG, so it is present in
    # hist slot m for m >= t_e+_LAG; capture slot is m = L+1.
    logsum = np.where(tevt + _LAG <= (L + 1)[None, :],
                      -np.log(sinv), 0.0).sum(0)
    featT_val = np.where(
        L < _S,
        feas[np.arange(_B), np.minimum(L, _S - 1), _STOP].astype(np.float64) - c,
        -c,
    )
    norm = c * L + logsum + np.log(hist63[L + 1, np.arange(_B)]) - featT_val

    # ---- gold score ----
    dt = np.float32
    pos = np.arange(_S + 2)
    lbl = np.concatenate(
        [np.full((_B, 1), _START, tag.dtype), tag, np.full((_B, 1), _STOP, tag.dtype)],
        axis=1,
    )
    lbl = np.where(pos[None, :] <= L[:, None], lbl, _STOP)
    trn = transitions[lbl[:, 1:], lbl[:, :-1]]
    tmask = (np.arange(_S + 1)[None, :] <= L[:, None]).astype(dt)
    trans_score = (trn.astype(dt) * tmask).sum(1)
    emit = np.take_along_axis(feas, tag[..., None], axis=2)[..., 0]
    emask = (np.arange(_S)[None, :] < L[:, None]).astype(dt)
    emit_score = (emit.astype(dt) * emask).sum(1)

    return (norm - (trans_score + emit_score)).astype(np.float32)


# revision 7
# speedup vs baseline: 1.0100x; 1.0100x over previous
"""CRF loss kernel for Trainium2 (8 NeuronCores, data-parallel over batch).

Math: the log-domain forward recurrence
    alpha_t[i] = logsumexp_j(alpha_{t-1}[j] + trans[i,j]) + feat_t[i]
is run in probability domain:
    P_t = exp(feat_t - c) * (E @ P_{t-1}),   E = exp(trans)
so each step is one tiny matmul plus one VectorE multiply.

Layout: T=64 tags use only half the 128 SBUF partitions, and the cost of a
DVE/matmul op depends only on its FREE size, so two 32-column batch groups
are STACKED on the partition axis (partitions 0:64 = cols 0:16, 64:128 =
cols 16:32 of the group) and advanced by a block-diagonal 128x128 transition
matrix. One matmul + one DVE multiply then serve 32 batch columns at free
width 16. Two such superchains interleave; steady state is DVE-bound at
~284ns/step = 2 x (125ns PSUM-access bubble + 16x1.04ns), just under the
serial link latency (~100ns matmul visibility + 45ns DVE decode + DVE busy).

All matmul operands are bf16 (error budget is huge: tolerance 2e-2 on a
loss of magnitude ~1e3; bf16 shares fp32's exponent range). A constant
shift c plus a per-superchain renorm every 128 steps keeps P in range;
renorms are staggered across superchains, measured off the critical path
(TensorE 2-row ones-matmul + DVE reciprocal), broadcast back to 128 rows
via a second matmul (2-partition stationary), copied to SBUF by the Act
engine, and folded into the exp(feat) tile of a later step by the Pool
engine. The STOP rows (partitions 63 and 127) of the state history are
archived in SBUF, streamed to DRAM in blocks behind the chain, and the
host epilogue picks slot seq_len[b]+1 per batch column.
"""
import numpy as np

_B, _S, _T = 512, 512, 64
_NCORE = 8
_BC = _B // _NCORE          # 64 batches per core
_G = 2                      # superchains per core
_WG = _BC // _G             # 32 batch columns per superchain
_R2 = 2                     # column groups stacked per superchain
_W = _WG // _R2             # 16 free columns per superchain tile
_P = 128                    # partitions
_START, _STOP = 62, 63
_R = 128                    # renorm period (per superchain)
_LAG = 4                    # renorm measured at t applies at t+_LAG
_NSTEP = _S + 1             # matmul steps 1..513
_NHIST = _NSTEP + 1         # history slots 0..513 (slot 0 unused)
# superchain g renorms at t = _R*(k+1) + 8*g: staggered so aux tiles don't
# coexist
_RENORM_TG = [[_R * (k + 1) + 8 * g for k in range(3)] for g in range(_G)]
_NEVT = 3
_CH = 8                     # steps per feat DMA/exp chunk
_HBLK = 4 * _CH             # hist63 streaming block (steps)

# boot column layout: [E2T | ones2 | Bm | featT2 steps 0..2]
_BOOT_E = 0                  # E2T at cols 0:128
_BOOT_ONES = _P              # ones2 at cols 128:130
_BOOT_F = _P + 2             # featT2[:, 0:96] at cols 130:226
_BOOT_COLS = _BOOT_F + 3 * _G * _W

_cache = {}


def _build_nc():
    import concourse.bass as bass
    import concourse.bacc as bacc
    import concourse.tile as tile
    from concourse import mybir
    from contextlib import ExitStack

    f32 = mybir.dt.float32
    bf16 = mybir.dt.bfloat16
    GW = _G * _W            # 32 = feature columns per step
    nc = bacc.Bacc("TRN2", target_bir_lowering=False, debug=False,
                   num_devices=_NCORE)
    # featT2 block m (m>=1) is step m's features; block 0 is p0.
    featT2 = nc.dram_tensor("featT2", [_P, (_NSTEP + 1) * GW], bf16,
                            kind="ExternalInput").ap()
    boot = nc.dram_tensor("boot", [_P, _BOOT_COLS], bf16,
                          kind="ExternalInput").ap()
    bootf = nc.dram_tensor("bootf", [_R2, _P], f32,
                           kind="ExternalInput").ap()
    hist63o = [nc.dram_tensor(f"hist63_{g}", [_R2, _NHIST * _W], bf16,
                              kind="ExternalOutput").ap() for g in range(_G)]
    sinvo = [nc.dram_tensor(f"sinv_{g}", [_R2, _NEVT * _W], f32,
                            kind="ExternalOutput").ap() for g in range(_G)]

    with tile.TileContext(nc) as tc, ExitStack() as ctx:
        consts = ctx.enter_context(tc.tile_pool(name="consts", bufs=1))
        fpool = ctx.enter_context(tc.tile_pool(name="fpool", bufs=3))
        epool = ctx.enter_context(tc.tile_pool(name="epool", bufs=3))
        ps_g = [ctx.enter_context(
            tc.tile_pool(name=f"ps{g}", bufs=2, space="PSUM"))
            for g in range(_G)]
        ps_aux = ctx.enter_context(tc.tile_pool(name="ps_aux", bufs=2,
                                                space="PSUM"))
        bcpool = ctx.enter_context(tc.tile_pool(name="bcpool", bufs=2))

        # one boot DMA delivers E2T, ones2, Bm, p0, and steps 1-2's features
        boot_sb = consts.tile([_P, _BOOT_COLS], bf16, name="boot_sb")
        nc.sync.dma_start(boot_sb[:, :], boot)
        E2_sb = boot_sb[:, _BOOT_E:_BOOT_E + _P]
        ones2 = boot_sb[:, _BOOT_ONES:_BOOT_ONES + _R2]
        Bm_sb = consts.tile([_R2, _P], f32, name="Bm_sb")
        nc.sync.dma_start(Bm_sb[:, :], bootf)
        Bm = Bm_sb[:, :]

        hists = [consts.tile([_P, _NHIST * _W], bf16, name=f"hist{g}")
                 for g in range(_G)]
        sinvs = [consts.tile([_R2, _NEVT * _W], f32, name=f"sinv{g}")
                 for g in range(_G)]

        renorm_at = {}
        fold_at = {}
        for g in range(_G):
            for e, te in enumerate(_RENORM_TG[g]):
                renorm_at[(te, g)] = e
                fold_at[(te + _LAG, g)] = e
        last_evt_t = max(te for g in range(_G) for te in _RENORM_TG[g])

        curs = [boot_sb[:, _BOOT_F + g * _W:_BOOT_F + (g + 1) * _W]
                for g in range(_G)]
        evt_bc = {}
        sinv_sent = False
        hist_sent = [_W] * _G            # hist63 cols streamed (slot 0 unused)
        t = 1
        while t <= _NSTEP:
            if t == 1:
                n_t = 2
                fch_f = boot_sb[:, _BOOT_F + GW:]
            else:
                n_t = min(_CH, _NSTEP - t + 1)
                fchunk = fpool.tile([_P, _CH * GW], bf16, tag="fchunk")
                nc.sync.dma_start(
                    fchunk[:, : n_t * GW],
                    featT2[:, t * GW: (t + n_t) * GW],
                )
                fch_f = fchunk[:, : n_t * GW]
            Fch = epool.tile([_P, _CH * GW], bf16, tag="Fch")
            nc.scalar.activation(
                Fch[:, : n_t * GW], fch_f,
                mybir.ActivationFunctionType.Exp,
            )
            for k in range(n_t):
                for g in range(_G):
                    fsl = Fch[:, k * GW + g * _W: k * GW + (g + 1) * _W]
                    if (t, g) in fold_at:
                        # apply the pending renorm scale to this step's F
                        bc = evt_bc.pop((fold_at[(t, g)], g))
                        nc.gpsimd.tensor_mul(fsl, fsl, bc[:, :])
                    ps = ps_g[g].tile([_P, _W], f32, tag=f"ps{g}")
                    nc.tensor.matmul(ps[:, :], E2_sb, curs[g],
                                     start=True, stop=True)
                    dst = hists[g][:, t * _W: (t + 1) * _W]
                    nc.vector.tensor_mul(dst, ps[:, :], fsl)
                    curs[g] = dst
                    if (t, g) in renorm_at:
                        e = renorm_at[(t, g)]
                        s_ps = ps_aux.tile([_R2, _W], f32, tag="s_ps")
                        nc.tensor.matmul(s_ps[:, :], ones2, dst,
                                         start=True, stop=True)
                        s_sb = bcpool.tile([_R2, _W], f32, tag="s_sb")
                        nc.scalar.copy(s_sb[:, :], s_ps[:, :])
                        sv = sinvs[g][:, e * _W: (e + 1) * _W]
                        nc.vector.reciprocal(sv, s_sb[:, :])
                        bc_ps = ps_aux.tile([_P, _W], f32, tag="bc_ps")
                        nc.tensor.matmul(bc_ps[:, :], Bm, sv,
                                         start=True, stop=True)
                        bc = bcpool.tile([_P, _W], f32, tag="bc")
                        nc.scalar.copy(bc[:, :], bc_ps[:, :])
                        evt_bc[(e, g)] = bc
                t += 1
            # stream completed hist63 blocks out behind the chain; flush every
            # chunk near the end so the final post-chain DMA is tiny
            if t <= _NSTEP and t - hist_sent[0] // _W > _HBLK:
                for g in range(_G):
                    lo, hi = hist_sent[g], t * _W
                    nc.sync.dma_start(hist63o[g][0:1, lo:hi],
                                      hists[g][_STOP:_STOP + 1, lo:hi])
                    nc.sync.dma_start(hist63o[g][1:2, lo:hi],
                                      hists[g][_T + _STOP:_T + _STOP + 1, lo:hi])
                    hist_sent[g] = hi
            if t > last_evt_t and not sinv_sent:
                sinv_sent = True
                for g in range(_G):
                    nc.sync.dma_start(sinvo[g], sinvs[g][:, :])
        # tail: flush through slot NSTEP-1 while the chain still runs, then
        # four 1-slot DMAs on four different DGE queues drain in parallel
        mid = _NSTEP * _W
        for g in range(_G):
            lo = hist_sent[g]
            nc.sync.dma_start(hist63o[g][0:1, lo:mid],
                              hists[g][_STOP:_STOP + 1, lo:mid])
            nc.scalar.dma_start(hist63o[g][1:2, lo:mid],
                                hists[g][_T + _STOP:_T + _STOP + 1, lo:mid])
        hi = (_NSTEP + 1) * _W
        tail_eng = [[nc.sync, nc.scalar], [nc.gpsimd, nc.sync]]
        for g in range(_G):
            tail_eng[g][0].dma_start(hist63o[g][0:1, mid:hi],
                                     hists[g][_STOP:_STOP + 1, mid:hi])
            tail_eng[g][1].dma_start(hist63o[g][1:2, mid:hi],
                                     hists[g][_T + _STOP:_T + _STOP + 1, mid:hi])
    nc.compile()
    return nc


def _prep_inputs(feas, transitions):
    import ml_dtypes

    E = np.exp(transitions.astype(np.float32))
    rows = np.ones(_T, bool)
    rows[_START] = False
    c = float(np.log(E.sum(1)[rows]).mean())
    ET = np.ascontiguousarray(E.T).astype(np.float32)  # ET[j,i]=E[i,j]
    E2T = np.zeros((_P, _P), np.float32)
    E2T[:_T, :_T] = ET
    E2T[_T:, _T:] = ET
    ones2 = np.zeros((_P, _R2), np.float32)
    ones2[:_T, 0] = 1.0
    ones2[_T:, 1] = 1.0
    Bpad = np.zeros((_P, _P), np.float32)       # rows 0:2 hold Bm
    Bpad[0, :_T] = 1.0
    Bpad[1, _T:] = 1.0

    # featT2 per core: [P, (1+NSTEP)*GW]; block 0 = p0 (stacked one-hot at
    # START), block m>=1 is step m's features shifted by -c; step 513 -> -c.
    ft = np.transpose(feas.astype(np.float32), (2, 1, 0)) - np.float32(c)  # [T,S,B]
    GW = _G * _W
    in_maps = []
    for cix in range(_NCORE):
        sl = ft[:, :, cix * _BC: (cix + 1) * _BC]                  # [T,S,BC]
        full = np.empty((_T, _NSTEP + 1, _BC), np.float32)
        p0 = np.zeros((_T, _BC), np.float32)
        p0[_START, :] = 1.0
        full[:, 0, :] = p0
        full[:, 1:_S + 1, :] = sl
        full[:, _S + 1, :] = -c
        # stack: featT2[p, m*GW + g*W + n] = full[p%T, m, g*WG + (p//T)*W + n]
        fu = full.reshape(_T, _NSTEP + 1, _G, _R2, _W)             # [T,m,g,r,n]
        featT2 = np.concatenate([fu[:, :, :, 0, :], fu[:, :, :, 1, :]],
                                axis=0)                            # [P,m,g,n]
        featT2 = np.ascontiguousarray(
            featT2.reshape(_P, (_NSTEP + 1) * GW)).astype(ml_dtypes.bfloat16)
        boot = np.hstack([
            E2T, ones2, featT2[:, 0:3 * GW].astype(np.float32),
        ]).astype(ml_dtypes.bfloat16)
        in_maps.append({
            "featT2": featT2,
            "boot": np.ascontiguousarray(boot),
            "bootf": np.ascontiguousarray(Bpad[0:_R2, :]),
        })
    return c, in_maps


def kernel(feas, transitions, tag, seq_len):
    from concourse.bass_utils import run_bass_kernel_spmd

    feas = np.asarray(feas)
    transitions = np.asarray(transitions)
    tag = np.asarray(tag)
    seq_len = np.asarray(seq_len)

    if "nc" not in _cache:
        _cache["nc"] = _build_nc()
    nc = _cache["nc"]

    c, in_maps = _prep_inputs(feas, transitions)
    res = run_bass_kernel_spmd(nc, in_maps, list(range(_NCORE))).results

    # ---- host epilogue: norm from archived history ----
    L = seq_len.astype(np.int64)                                        # [B]
    # batch column b = cix*BC + g*WG + r*W + n
    hist63 = np.concatenate(
        [res[cix][f"hist63_{g}"].reshape(_R2, _NHIST, _W)
         .transpose(1, 0, 2).reshape(_NHIST, _WG).astype(np.float64)
         for cix in range(_NCORE) for g in range(_G)], axis=1
    )                                                                   # [NHIST, B]
    sinv = np.concatenate(
        [res[cix][f"sinv_{g}"].reshape(_R2, _NEVT, _W)
         .transpose(1, 0, 2).reshape(_NEVT, _WG).astype(np.float64)
         for cix in range(_NCORE) for g in range(_G)], axis=1
    )                                                                   # [NEVT, B]
    tevt = np.concatenate(
        [np.asarray(_RENORM_TG[g])[:, None].repeat(_WG, 1)
         for _ in range(_NCORE) for g in range(_G)], axis=1
    )                                                                   # [NEVT, B]
    # scale 1/s_e is folded into F of step t_e+_LAG, so it is present in
    # hist slot m for m >= t_e+_LAG; capture slot is m = L+1.
    logsum = np.where(tevt + _LAG <= (L + 1)[None, :],
                      -np.log(sinv), 0.0).sum(0)
    featT_val = np.where(
        L < _S,
        feas[np.arange(_B), np.minimum(L, _S - 1), _STOP].astype(np.float64) - c,
        -c,
    )
    norm = c * L + logsum + np.log(hist63[L + 1, np.arange(_B)]) - featT_val

    # ---- gold score ----
    dt = np.float32
    pos = np.arange(_S + 2)
    lbl = np.concatenate(
        [np.full((_B, 1), _START, tag.dtype), tag, np.full((_B, 1), _STOP, tag.dtype)],
        axis=1,
    )
    lbl = np.where(pos[None, :] <= L[:, None], lbl, _STOP)
    trn = transitions[lbl[:, 1:], lbl[:, :-1]]
    tmask = (np.arange(_S + 1)[None, :] <= L[:, None]).astype(dt)
    trans_score = (trn.astype(dt) * tmask).sum(1)
    emit = np.take_along_axis(feas, tag[..., None], axis=2)[..., 0]
    emask = (np.arange(_S)[None, :] < L[:, None]).astype(dt)
    emit_score = (emit.astype(dt) * emask).sum(1)

    return (norm - (trans_score + emit_score)).astype(np.float32)


# revision 8
# speedup vs baseline: 3.3178x; 3.2851x over previous
"""CRF loss kernel for Trainium2 (8 NeuronCores, time-sharded).

Math: the log-domain forward recurrence
    alpha_t[i] = logsumexp_j(alpha_{t-1}[j] + trans[i,j]) + feat_t[i]
is run in probability domain:
    P_t = exp(feat_t - c) * (E @ P_{t-1}),   E = exp(trans)
so each step is one matmul plus one VectorE multiply.

Sharding: the per-step op cost is dominated by fixed per-instruction
overheads (125ns DVE PSUM-access bubble, ~100ns matmul latency), so batch
width is nearly free and the 513 serial steps are the wall. E and D_t=
diag(exp(feat)) are strictly positive, so the normalized state direction
contracts to the true one in a handful of steps (measured: 1e-5 direction
error after 8 warmup steps, 1e-10 after 16 - far below bf16 noise). Each
core therefore owns a 64-step time block over ALL 512 batch columns,
warm-starting 8 steps early from a uniform state; core 0 starts exactly
from p0. The host telescopes per-block log-norm growth factors (measured
by on-chip ones-matmul column sums at local steps 8 and 72) to recover
the exact log-partition value at each column's capture slot seq_len+1.

Layout: T=64 tags use half the 128 SBUF partitions, so two 256-column
groups are stacked on the partition axis (block-diagonal 128x128
transition matrix); per local step the state is [128, 256] split into 2
interleaved chains of 128 free columns. Steady state is DVE-bound at
~517ns/step = 2 x (125ns PSUM bubble + 128x1.04ns). All matmul operands
bf16. One renorm per core: 1/s_start folded into F at local step 12
keeps the capture values in bf16 range. Features for the first 26 local
steps ship pre-exponentiated in three parallel boot DMAs (SP/Pool/SP) so
the chain starts without waiting on the Act engine; later chunks are
exp'd on Act behind the chain. STOP rows (partitions 63/127) archive via
the history buffer itself, streamed out on the Pool/Act DGE queues.
"""
import numpy as np

_B, _S, _T = 512, 512, 64
_NCORE = 8
_P = 128
_START, _STOP = 62, 63
_WARM = 8                    # warmup steps (direction contraction)
_RLOC = 72                   # local steps per core
_TRUST = _RLOC - _WARM       # trusted steps per core (core 0: _RLOC)
_LAG = 4                     # fold of 1/s_start applies at _WARM+_LAG
_FREE = 256                  # free columns per step tile (512 cols / 2)
_CW = _FREE // 2             # 128 free columns per chain
_NBOOT = 26                  # locals 1.._NBOOT ship pre-exp'd
_NB1 = 10                    # boot carries 1..2; bootb1: 3..10; bootb2: 11..26

_cache = {}


def _build_nc():
    import concourse.bass as bass
    import concourse.bacc as bacc
    import concourse.tile as tile
    from concourse import mybir
    from contextlib import ExitStack

    f32 = mybir.dt.float32
    bf16 = mybir.dt.bfloat16
    nc = bacc.Bacc("TRN2", target_bir_lowering=False, debug=False,
                   num_devices=_NCORE)
    # boot: [E2T | ones2 | p0 | expF(1..2)]
    bcols = _P + 2 + 3 * _FREE
    boot = nc.dram_tensor("boot", [_P, bcols], bf16, kind="ExternalInput").ap()
    bootb1 = nc.dram_tensor("bootb1", [_P, (_NB1 - 2) * _FREE], bf16,
                            kind="ExternalInput").ap()
    bootb2 = nc.dram_tensor("bootb2", [_P, (_NBOOT - _NB1) * _FREE], bf16,
                            kind="ExternalInput").ap()
    bootf = nc.dram_tensor("bootf", [2, _P], f32, kind="ExternalInput").ap()
    # raw (not exp'd) features for locals _NBOOT+1.._RLOC
    featR = nc.dram_tensor("featR", [_P, (_RLOC - _NBOOT) * _FREE], bf16,
                           kind="ExternalInput").ap()
    histo = nc.dram_tensor("hist", [_P - _STOP, (_RLOC + 1) * _FREE], bf16,
                           kind="ExternalOutput").ap()
    sumso = nc.dram_tensor("sums", [2, 2 * _FREE], f32,
                           kind="ExternalOutput").ap()

    with tile.TileContext(nc) as tc, ExitStack() as ctx:
        consts = ctx.enter_context(tc.tile_pool(name="consts", bufs=1))
        fpool = ctx.enter_context(tc.tile_pool(name="fpool", bufs=3))
        epool = ctx.enter_context(tc.tile_pool(name="epool", bufs=3))
        ps_g = [ctx.enter_context(
            tc.tile_pool(name=f"ps{g}", bufs=2, space="PSUM"))
            for g in range(2)]
        ps_aux = ctx.enter_context(tc.tile_pool(name="ps_aux", bufs=2,
                                                space="PSUM"))

        boot_sb = consts.tile([_P, bcols], bf16, name="boot_sb")
        nc.sync.dma_start(boot_sb[:, :], boot)              # SP queue first
        b2_sb = consts.tile([_P, (_NBOOT - _NB1) * _FREE], bf16, name="b2_sb")
        nc.sync.dma_start(b2_sb[:, :], bootb2)              # SP second
        b1_sb = consts.tile([_P, (_NB1 - 2) * _FREE], bf16, name="b1_sb")
        nc.gpsimd.dma_start(b1_sb[:, :], bootb1)            # Pool queue
        Bm_sb = consts.tile([2, _P], f32, name="Bm_sb")
        nc.scalar.dma_start(Bm_sb[:, :], bootf)             # Act queue

        E2_sb = boot_sb[:, 0:_P]
        ones2 = boot_sb[:, _P:_P + 2]
        p0 = boot_sb[:, _P + 2:_P + 2 + _FREE]

        hist = consts.tile([_P, (_RLOC + 1) * _FREE], bf16, name="hist")
        sums_sb = consts.tile([2, 2 * _FREE], f32, name="sums_sb")
        sv_sb = consts.tile([2, _FREE], f32, name="sv_sb")
        bc_sb = consts.tile([_P, _FREE], f32, name="bc_sb")

        # raw-feature chunks (exp'd on Act) for locals _NBOOT+1.._RLOC
        chunks = [(27, 34), (35, 50), (51, 66), (67, 72)]
        chunk_iter = iter(chunks)
        state = {"l0": None, "e": None}

        def fsrc(l):
            """F tile (exp'd, bf16) for local step l, from boot or chunk."""
            if l <= 2:
                off = _P + 2 + l * _FREE
                return boot_sb[:, off:off + _FREE]
            if l <= _NB1:
                off = (l - 3) * _FREE
                return b1_sb[:, off:off + _FREE]
            if l <= _NBOOT:
                off = (l - _NB1 - 1) * _FREE
                return b2_sb[:, off:off + _FREE]
            off = (l - state["l0"]) * _FREE
            return state["e"][:, off:off + _FREE]

        next_chunk = next(chunk_iter)
        curs = [p0[:, g * _CW:(g + 1) * _CW] for g in range(2)]
        # archive blocks: last_slot -> (engine, first_slot)
        arch_plan = {24: ('pool', 1), 48: ('pool', 25), 60: ('act', 49),
                     70: ('act', 61), 72: ('pool', 71)}

        for l in range(1, _RLOC + 1):
            if next_chunk is not None and l == next_chunk[0]:
                lo, hi = next_chunk
                n = hi - lo + 1
                fch = fpool.tile([_P, 16 * _FREE], bf16, tag="fch")
                nc.sync.dma_start(fch[:, :n * _FREE],
                                  featR[:, (lo - _NBOOT - 1) * _FREE:
                                        (hi - _NBOOT) * _FREE])
                ech = epool.tile([_P, 16 * _FREE], bf16, tag="ech")
                nc.scalar.activation(ech[:, :n * _FREE], fch[:, :n * _FREE],
                                     mybir.ActivationFunctionType.Exp)
                state["l0"], state["e"] = lo, ech
                next_chunk = next(chunk_iter, None)
            f_l = fsrc(l)
            if l == _WARM + _LAG:
                # fold 1/s_start into this step's F (Pool, off critical path)
                nc.gpsimd.tensor_mul(f_l, f_l, bc_sb[:, :])
            for g in range(2):
                fsl = f_l[:, g * _CW:(g + 1) * _CW]
                ps = ps_g[g].tile([_P, _CW], f32, tag=f"ps{g}")
                nc.tensor.matmul(ps[:, :], E2_sb, curs[g],
                                 start=True, stop=True)
                dst = hist[:, l * _FREE + g * _CW:
                           l * _FREE + (g + 1) * _CW]
                nc.vector.tensor_mul(dst, ps[:, :], fsl)
                curs[g] = dst
            if l == _WARM:
                # s_start: per-column sums via 2-row ones matmul
                s_ps = ps_aux.tile([2, _FREE], f32, tag="s_ps")
                nc.tensor.matmul(s_ps[:, :], ones2,
                                 hist[:, l * _FREE:(l + 1) * _FREE],
                                 start=True, stop=True)
                nc.scalar.copy(sums_sb[:, 0:_FREE], s_ps[:, :])
                nc.vector.reciprocal(sv_sb[:, :], sums_sb[:, 0:_FREE])
                bc_ps = ps_aux.tile([_P, _FREE], f32, tag="bc_ps")
                nc.tensor.matmul(bc_ps[:, :], Bm_sb[:, :], sv_sb[:, :],
                                 start=True, stop=True)
                nc.scalar.copy(bc_sb[:, :], bc_ps[:, :])
            if l == _RLOC:
                s_ps2 = ps_aux.tile([2, _FREE], f32, tag="s_ps")
                nc.tensor.matmul(s_ps2[:, :], ones2,
                                 hist[:, l * _FREE:(l + 1) * _FREE],
                                 start=True, stop=True)
                nc.scalar.copy(sums_sb[:, _FREE:2 * _FREE], s_ps2[:, :])
                nc.sync.dma_start(sumso, sums_sb[:, :])
            if l in arch_plan:
                eng_name, lo = arch_plan[l]
                eng = nc.gpsimd if eng_name == 'pool' else nc.scalar
                eng.dma_start(histo[:, lo * _FREE:(l + 1) * _FREE],
                              hist[_STOP:_P, lo * _FREE:(l + 1) * _FREE])
    nc.compile()
    return nc


def _prep_inputs(feas, transitions):
    import ml_dtypes
    bf = ml_dtypes.bfloat16

    E = np.exp(transitions.astype(np.float32))
    rows = np.ones(_T, bool)
    rows[_START] = False
    c = float(np.log(E.sum(1)[rows]).mean())
    ET = np.ascontiguousarray(E.T).astype(np.float32)       # ET[j,i]=E[i,j]
    E2T = np.zeros((_P, _P), np.float32)
    E2T[:_T, :_T] = ET
    E2T[_T:, _T:] = ET
    ones2 = np.zeros((_P, 2), np.float32)
    ones2[:_T, 0] = 1.0
    ones2[_T:, 1] = 1.0
    Bm = np.zeros((2, _P), np.float32)
    Bm[0, :_T] = 1.0
    Bm[1, _T:] = 1.0

    # stacked raw features per local step: stk[p, l, n] =
    #   feat[p%64, base+l-1, (p//64)*256 + n] - c   (pad -c past S)
    ft = np.transpose(feas.astype(np.float32), (2, 1, 0)) - np.float32(c)
    # ft: [T, S, B]
    in_maps = []
    for cix in range(_NCORE):
        base = 64 * cix
        stk = np.full((_P, _RLOC + 1, _FREE), -c, np.float32)
        n_real = min(_S - base, _RLOC)              # locals with real feats
        sl = ft[:, base:base + n_real, :]           # [T, n, B]
        stk[:_T, 1:n_real + 1, :] = np.ascontiguousarray(
            sl[:, :, 0:_FREE])
        stk[_T:, 1:n_real + 1, :] = np.ascontiguousarray(
            sl[:, :, _FREE:2 * _FREE])
        stk_bf = stk.astype(bf)
        expF = np.exp(stk_bf.astype(np.float32)).astype(bf)  # [P, l, FREE]
        if cix == 0:
            p0 = np.zeros((_P, _FREE), np.float32)
            p0[_START, :] = 1.0
            p0[_T + _START, :] = 1.0
        else:
            p0 = np.full((_P, _FREE), 1.0 / _T, np.float32)
        boot = np.hstack([
            E2T, ones2, p0,
            expF[:, 1:3, :].reshape(_P, 2 * _FREE).astype(np.float32),
        ]).astype(bf)
        in_maps.append({
            "boot": np.ascontiguousarray(boot),
            "bootb1": np.ascontiguousarray(
                expF[:, 3:_NB1 + 1, :].reshape(_P, -1)),
            "bootb2": np.ascontiguousarray(
                expF[:, _NB1 + 1:_NBOOT + 1, :].reshape(_P, -1)),
            "bootf": np.ascontiguousarray(Bm),
            "featR": np.ascontiguousarray(
                stk_bf[:, _NBOOT + 1:, :].reshape(_P, -1)),
        })
    return c, in_maps


def kernel(feas, transitions, tag, seq_len):
    from concourse.bass_utils import run_bass_kernel_spmd

    feas = np.asarray(feas)
    transitions = np.asarray(transitions)
    tag = np.asarray(tag)
    seq_len = np.asarray(seq_len)

    if "nc" not in _cache:
        _cache["nc"] = _build_nc()
    nc = _cache["nc"]

    c, in_maps = _prep_inputs(feas, transitions)
    res = run_bass_kernel_spmd(nc, in_maps, list(range(_NCORE))).results

    # ---- host epilogue: telescoped norm from per-core archives ----
    L = seq_len.astype(np.int64)                                      # [B]
    nrow = _P - _STOP
    # stops[j, l, b]: archived STOP value; col b -> (row 0 | row 64, n=b%256)
    stops = np.zeros((_NCORE, _RLOC + 1, _B))
    s_start = np.zeros((_NCORE, _B))
    s_end = np.zeros((_NCORE, _B))
    for j in range(_NCORE):
        h = res[j]["hist"].reshape(nrow, _RLOC + 1, _FREE)
        stops[j, :, 0:_FREE] = h[0].astype(np.float64)
        stops[j, :, _FREE:2 * _FREE] = h[_T].astype(np.float64)
        sm = res[j]["sums"].reshape(2, 2, _FREE).astype(np.float64)
        s_start[j, 0:_FREE] = sm[0, 0]
        s_start[j, _FREE:] = sm[1, 0]
        s_end[j, 0:_FREE] = sm[0, 1]
        s_end[j, _FREE:] = sm[1, 1]

    growth = np.log(s_end)                              # [NCORE, B]
    growth[0] += np.log(s_start[0])                     # core 0: undo fold
    prefix = np.concatenate([np.zeros((1, _B)), np.cumsum(growth, 0)], 0)

    m = L + 1                                           # capture slot
    K = np.where(m <= _RLOC, 0, (m - _RLOC - 1) // _TRUST + 1)
    lloc = m - 64 * K
    bb = np.arange(_B)
    C_raw = np.log(stops[K, lloc, bb])
    addback = np.where(lloc >= _WARM + _LAG, np.log(s_start[K, bb]), 0.0)
    lvalue = np.where(
        K == 0,
        C_raw + addback,
        C_raw + addback + prefix[K, bb] - np.log(s_start[K, bb]),
    )
    featT_val = np.where(
        L < _S,
        feas[bb, np.minimum(L, _S - 1), _STOP].astype(np.float64) - c,
        -c,
    )
    norm = c * L + lvalue - featT_val

    # ---- gold score ----
    dt = np.float32
    pos = np.arange(_S + 2)
    lbl = np.concatenate(
        [np.full((_B, 1), _START, tag.dtype), tag,
         np.full((_B, 1), _STOP, tag.dtype)], axis=1,
    )
    lbl = np.where(pos[None, :] <= L[:, None], lbl, _STOP)
    trn = transitions[lbl[:, 1:], lbl[:, :-1]]
    tmask = (np.arange(_S + 1)[None, :] <= L[:, None]).astype(dt)
    trans_score = (trn.astype(dt) * tmask).sum(1)
    emit = np.take_along_axis(feas, tag[..., None], axis=2)[..., 0]
    emask = (np.arange(_S)[None, :] < L[:, None]).astype(dt)
    emit_score = (emit.astype(dt) * emask).sum(1)

    return (norm - (trans_score + emit_score)).astype(np.float32)
